# revision 13
# baseline (speedup 1.0000x reference)
"""ArcFace (AngularPenaltySMLoss) fused loss kernel for 8 Trainium2 NeuronCores.

Strategy: data-parallel over rows N (each core owns N/8 = 1024 rows of x and
target, streams the full W). Per core, fully fused on-chip:
  1. row-normalize x (DVE + ACT sqrt), build xn^T (PE transposes, bf16)
  2. stream W in 1536-column tiles: natural load -> bf16 cast -> PE transpose
     -> matmul (bf16, PSUM f32) -> ACT exp(scale=S) with accum_out row-sums.
     logits never touch HBM.
  3. target score t_i = xn[i].xW[target_i] via indirect-DMA row gather + DVE dot
  4. numerator via cos(acos(t)+M) = t*cosM - sinM*sqrt(1-t^2)  (no arccos)
  5. per-core partial sum of L_i; host combines 8 scalars: loss = -sum/8192
"""

import math

import numpy as np

S = 30.0
MARGIN = 0.3
EPS = 1e-7
N, D, C = 8192, 256, 10000
NCORES = 8
NLOC = N // NCORES  # 1024 rows per core
NJ = NLOC // 128  # 8 row-chunks of 128 partitions
CT = 1536  # class-tile width per main-loop round
NR = math.ceil(C / CT)  # 7 rounds (6*1536 + 784)

_CACHE = {}


def _build():
    import concourse.bass as bass
    import concourse.mybir as mybir
    import concourse.tile as tile
    from concourse import bacc
    from concourse.masks import make_identity

    f32 = mybir.dt.float32
    bf16 = mybir.dt.bfloat16
    i32 = mybir.dt.int32
    AF = mybir.ActivationFunctionType
    OP = mybir.AluOpType

    nc = bacc.Bacc()
    x_ext = nc.declare_dram_parameter("x", [NLOC, D], f32, isOutput=False)
    w_ext = nc.declare_dram_parameter("w", [C, D], f32, isOutput=False)
    t_ext = nc.declare_dram_parameter("tgt", [NLOC, 1], i32, isOutput=False)
    out_ext = nc.declare_dram_parameter("out", [1, 1], f32, isOutput=True)

    with tile.TileContext(nc) as tc:
        with (
            tc.tile_pool(name="singles", bufs=1) as singles,
            tc.tile_pool(name="wpool", bufs=7) as wpool,
            tc.tile_pool(name="wtpool", bufs=7) as wtpool,
            tc.tile_pool(name="pmain", bufs=2, space="PSUM") as psum_main,
            tc.tile_pool(name="ptp", bufs=1, space="PSUM") as psum_tp,
        ):
            ident = singles.tile([128, 128], bf16)
            make_identity(nc, ident)

            # ---------------- phase 0: load x, normalize rows ----------------
            xt = singles.tile([128, NJ, D], f32)
            nc.sync.dma_start(
                out=xt, in_=x_ext[:, :].rearrange("(j p) d -> p j d", p=128)
            )
            xsq = singles.tile([128, NJ, D], f32)
            nc.vector.tensor_tensor(
                out=xsq, in0=xt, in1=xt, op=OP.mult
            )
            ss = singles.tile([128, NJ], f32)
            nc.vector.tensor_reduce(
                out=ss, in_=xsq, axis=mybir.AxisListType.X, op=OP.add
            )
            nrm = singles.tile([128, NJ], f32)
            nc.scalar.activation(out=nrm, in_=ss, func=AF.Sqrt)
            rinv = singles.tile([128, NJ], f32)
            nc.vector.reciprocal(out=rinv, in_=nrm)
            xn = singles.tile([128, NJ, D], f32)
            for j in range(NJ):
                nc.vector.tensor_scalar(
                    out=xn[:, j, :],
                    in0=xt[:, j, :],
                    scalar1=rinv[:, j : j + 1],
                    scalar2=None,
                    op0=OP.mult,
                )
            xnb = singles.tile([128, NJ, D], bf16)
            nc.vector.tensor_copy(out=xnb, in_=xn)

            # xn^T (stationary operand): two d-chunks of [128, 1024] bf16
            xnT = singles.tile([128, 2, NLOC], bf16)
            for dc in range(2):
                ptp0 = psum_tp.tile([128, NLOC], bf16, tag="tp")
                for j in range(NJ):
                    nc.tensor.transpose(
                        out=ptp0[:, j * 128 : (j + 1) * 128],
                        in_=xnb[:, j, dc * 128 : (dc + 1) * 128],
                        identity=ident,
                    )
                nc.vector.tensor_copy(out=xnT[:, dc, :], in_=ptp0)

            # ---------------- phase 1: target gather + numerator ----------------
            tg = singles.tile([128, NJ], i32)
            nc.sync.dma_start(
                out=tg, in_=t_ext[:, :].rearrange("(j p) o -> p (j o)", p=128)
            )
            wg = singles.tile([128, NJ, D], f32)
            for j in range(NJ):
                nc.gpsimd.indirect_dma_start(
                    out=wg[:, j, :],
                    out_offset=None,
                    in_=w_ext[:, :],
                    in_offset=bass.IndirectOffsetOnAxis(ap=tg[:, j : j + 1], axis=0),
                )
            traw = singles.tile([128, NJ], f32)
            prod = singles.tile([128, NJ, D], f32)
            for j in range(NJ):
                nc.vector.tensor_tensor(
                    out=prod[:, j, :], in0=xn[:, j, :], in1=wg[:, j, :], op=OP.mult
                )
            nc.vector.tensor_reduce(
                out=traw, in_=prod, axis=mybir.AxisListType.X, op=OP.add
            )
            tcl = singles.tile([128, NJ], f32)
            nc.vector.tensor_scalar(
                out=tcl,
                in0=traw,
                scalar1=-1.0 + EPS,
                scalar2=1.0 - EPS,
                op0=OP.max,
                op1=OP.min,
            )
            usq = singles.tile([128, NJ], f32)  # 1 - t^2
            t2 = singles.tile([128, NJ], f32)
            nc.vector.tensor_tensor(out=t2, in0=tcl, in1=tcl, op=OP.mult)
            nc.vector.tensor_scalar(
                out=usq, in0=t2, scalar1=-1.0, scalar2=1.0, op0=OP.mult, op1=OP.add
            )
            rt = singles.tile([128, NJ], f32)  # sqrt(1-t^2)
            nc.scalar.activation(out=rt, in_=usq, func=AF.Sqrt)
            numer = singles.tile([128, NJ], f32)
            tcos = singles.tile([128, NJ], f32)
            nc.vector.tensor_scalar(
                out=tcos, in0=tcl, scalar1=S * math.cos(MARGIN), scalar2=None,
                op0=OP.mult,
            )
            rtm = singles.tile([128, NJ], f32)
            nc.vector.tensor_scalar(
                out=rtm, in0=rt, scalar1=-S * math.sin(MARGIN), scalar2=None,
                op0=OP.mult,
            )
            nc.vector.tensor_tensor(out=numer, in0=rtm, in1=tcos, op=OP.add)

            # ---------------- phase 2: main loop over class tiles ----------------
            acc = singles.tile([128, NJ, 8], f32)
            nc.vector.memset(acc, 0.0)
            expdump = singles.tile([128, CT], bf16)

            for r in range(NR):
                c0 = r * CT
                cw = min(CT, C - c0)
                nblk = math.ceil(cw / 128)
                nfull = cw // 128
                rem = cw - nfull * 128

                # SWDGE DMA with inline f32->bf16 cast (HWDGE can't cast and
                # its pseudo-DMA only supports a single sync wait).
                wnb = wpool.tile([128, 12, D], bf16, tag="wnb")
                if nfull > 0:
                    nc.gpsimd.dma_start(
                        out=wnb[:, :nfull, :],
                        in_=w_ext[c0 : c0 + nfull * 128, :].rearrange(
                            "(a p) d -> p a d", p=128
                        ),
                    )
                if rem > 0:
                    nc.gpsimd.dma_start(
                        out=wnb[0:rem, nfull, :],
                        in_=w_ext[c0 + nfull * 128 : c0 + cw, :],
                    )

                wt = wtpool.tile([128, 2, CT], bf16, tag="wt")
                for dc in range(2):
                    ptp = psum_tp.tile([128, CT], bf16, tag="tp")
                    for a in range(nblk):
                        rows_a = min(128, cw - a * 128)
                        nc.tensor.transpose(
                            out=ptp[:, a * 128 : a * 128 + rows_a],
                            in_=wnb[0:rows_a, a, dc * 128 : (dc + 1) * 128],
                            identity=ident[0:rows_a, 0:rows_a],
                        )
                    nc.vector.tensor_copy(out=wt[:, dc, :cw], in_=ptp[:, :cw])

                nsub = math.ceil(cw / 512)
                for j in range(NJ):
                    pm = psum_main.tile([128, CT], f32, tag="pm")
                    for dc in range(2):
                        for s_ in range(nsub):
                            sw = min(512, cw - s_ * 512)
                            nc.tensor.matmul(
                                out=pm[:, s_ * 512 : s_ * 512 + sw],
                                lhsT=xnT[:, dc, j * 128 : (j + 1) * 128],
                                rhs=wt[:, dc, s_ * 512 : s_ * 512 + sw],
                                start=(dc == 0),
                                stop=(dc == 1),
                                skip_group_check=True,
                            )
                    nc.scalar.activation(
                        out=expdump[:, :cw],
                        in_=pm[:, :cw],
                        func=AF.Exp,
                        scale=S,
                        accum_out=acc[:, j, r : r + 1],
                    )

            # ---------------- phase 3: combine ----------------
            exp_num = singles.tile([128, NJ], f32)
            nc.scalar.activation(out=exp_num, in_=numer, func=AF.Exp)
            exp_st = singles.tile([128, NJ], f32)
            nc.scalar.activation(out=exp_st, in_=tcl, func=AF.Exp, scale=S)
            rowsum = singles.tile([128, NJ], f32)
            nc.vector.tensor_reduce(
                out=rowsum, in_=acc, axis=mybir.AxisListType.X, op=OP.add
            )
            dtmp = singles.tile([128, NJ], f32)
            nc.vector.tensor_tensor(out=dtmp, in0=rowsum, in1=exp_num, op=OP.add)
            denom = singles.tile([128, NJ], f32)
            nc.vector.tensor_tensor(out=denom, in0=dtmp, in1=exp_st, op=OP.subtract)
            logd = singles.tile([128, NJ], f32)
            nc.scalar.activation(out=logd, in_=denom, func=AF.Ln)
            Lt = singles.tile([128, NJ], f32)
            nc.vector.tensor_tensor(out=Lt, in0=numer, in1=logd, op=OP.subtract)
            Lrow = singles.tile([128, 1], f32)
            nc.vector.tensor_reduce(
                out=Lrow, in_=Lt, axis=mybir.AxisListType.X, op=OP.add
            )
            ones = singles.tile([128, 1], f32)
            nc.vector.memset(ones, 1.0)
            psum_s = psum_tp.tile([1, 1], f32, tag="tp")
            nc.tensor.matmul(
                out=psum_s, lhsT=Lrow, rhs=ones, start=True, stop=True
            )
            Lp = singles.tile([1, 1], f32)
            nc.vector.tensor_copy(out=Lp, in_=psum_s)
            nc.sync.dma_start(out=out_ext[:, :], in_=Lp)

    nc.finalize()  # Bacc.compile(): reg alloc + sync-wait legalization
    return nc


def _get_nc():
    if "nc" not in _CACHE:
        _CACHE["nc"] = _build()
    return _CACHE["nc"]


def kernel(x, W, target):
    from concourse.bass_utils import run_bass_kernel_spmd

    x = np.ascontiguousarray(np.asarray(x), dtype=np.float32)
    W = np.ascontiguousarray(np.asarray(W), dtype=np.float32)
    tgt = np.ascontiguousarray(np.asarray(target).astype(np.int32).reshape(N, 1))

    nc = _get_nc()
    in_maps = [
        {
            "x": x[c * NLOC : (c + 1) * NLOC],
            "w": W,
            "tgt": tgt[c * NLOC : (c + 1) * NLOC],
        }
        for c in range(NCORES)
    ]
    res = run_bass_kernel_spmd(nc, in_maps, core_ids=list(range(NCORES)))
    parts = np.stack([res.results[i]["out"].reshape(()) for i in range(NCORES)])
    total = np.sum(parts, dtype=np.float32)
    return np.float32(-(total / np.float32(N)))
